# revision 55
# baseline (speedup 1.0000x reference)
"""NetVLAD Trainium2 kernel — data-parallel over N across 8 cores, bf16.

Per core: 4 images [C=128, P=4096], host-cast to bf16.  x is loaded twice
per image: plain [c, p] (logits stationaries) and via one HWDGE
DMA-transpose [p, c] straight into SBUF (no PE transpose, no psum evict).

Per-image preamble (pipelined one image ahead):
  PE:   ssq via 32 ones-column matmuls (stationary = xsq tile from ACT
        Square of xin) -> psum [p, 32 cols]; decoupled from the transpose.
  DVE:  invn via Quake rsqrt (int bitcast + shift + 2 Newton steps) so
        exp is the ONLY ACT table function -> a single ACT_TABLE_LOAD for
        the whole kernel; n = ssq*invn lands directly in the xTs 129th
        column (stride-144 tiles, 32B-aligned).
Per 1024-px chunk (8 tiles):
  PE:   8 logits matmuls (stationary x_tile bf16, moving wT 64 cols).
  DVE:  lu = raw*invn (from psum); -max reduce (negate=True); sumexp
        reduce; reciprocal; rcol.
  Pool: ll = lu + b512; dd = ll - m (broadcast); a_r = ee * rcol.
  ACT:  ee = exp(dd) as one big [p,512] op.
  PE:   8 vlad matmuls (stationary a_r[:, :56], moving [xT | n] 129 cols),
        accumulated into psum [56, 129]; emission deferred one chunk so
        the in-order PE queue always has independent logits work first.
Batched tails: vk = term1 - s*cen, PE transpose, intra-norm via Quake
rsqrt * 1/sqrt(128) (the global norm is exactly 1/sqrt(128) because every
intra-normalized column has unit norm), transpose back, DMA.
"""

import sys

for _p in ("/opt/trn_rl_repo",):
    if _p not in sys.path:
        sys.path.insert(0, _p)

import numpy as np
import ml_dtypes

NIMG = 4      # images per core
C = 128
K = 64
KE = 56
P = 4096
TPC = 8       # 128-px tiles per chunk
CH = TPC * 128
NCH = P // CH  # 4 chunks per image
TS = 144      # xTs per-tile stride (bf16 elems); 288B = 32B-aligned

_cache = {}


def _build():
    import concourse.mybir as mybir
    from concourse import bacc, tile

    f32 = mybir.dt.float32
    i32 = mybir.dt.int32
    bf16 = mybir.dt.bfloat16
    Alu = mybir.AluOpType
    Act = mybir.ActivationFunctionType

    nc = bacc.Bacc()
    x_in = nc.declare_dram_parameter("x", [NIMG, C, P], bf16, isOutput=False)
    cb_in = nc.declare_dram_parameter("cstb", [C, K], bf16, isOutput=False)
    # f32 consts: b64 [0:64] | identF [64:192] | cen rows0:56 [192:320]
    # | ln(1/sqrt(128)) [320]
    cf_in = nc.declare_dram_parameter("cstf", [C, 840], f32, isOutput=False)
    out_ext = nc.declare_dram_parameter("out", [NIMG, KE, C], f32, isOutput=True)
    dbg_ext = nc.declare_dram_parameter("dbg", [C, 704], f32, isOutput=True)

    with tile.TileContext(nc) as tc:
        with (
            tc.tile_pool(name="const", bufs=1) as cpool,
            tc.tile_pool(name="xin", bufs=2) as xpool,
            tc.tile_pool(name="xts", bufs=2) as tpool,
            tc.tile_pool(name="work", bufs=5) as wpool,
            tc.tile_pool(name="stats", bufs=8) as spool,
            tc.tile_pool(name="fin", bufs=2) as fpool,
            tc.tile_pool(name="psL", bufs=2, space="PSUM") as pL,
            tc.tile_pool(name="psV", bufs=4, space="PSUM") as pV,
            tc.tile_pool(name="psS", bufs=2, space="PSUM") as pS,
        ):
            wT = cpool.tile([C, K], bf16, tag="wT")
            onescb = cpool.tile([C, 1], bf16, tag="onescb")
            cstf = cpool.tile([C, 840], f32, tag="cstf")
            nc.sync.dma_start(wT[:], cb_in[:])
            nc.vector.memset(onescb[:], 1.0)
            magic = cpool.tile([C, NCH * TPC], i32, tag="magic")
            nc.vector.memset(magic[:], 0x5f3759df)
            onesb = onescb[:]
            nc.sync.dma_start(cstf[:], cf_in[:])
            b64 = cstf[:, 0:K]
            identF = cstf[:, 64:192]
            cen = cstf[0:KE, 192:320]
            gnl = cstf[:, 320:321]
            b512 = cstf[:, 328:328 + TPC * K]

            PT = NCH * TPC  # 32 pixel tiles per image

            xTs_l, invc_l, xin_l, psV_l = [], [], [], []
            pend_vlad = []

            psS_l = []

            def preamble(img):
                xTs = tpool.tile([C, PT * TS], bf16, tag="xTs",
                                 name=f"xTs{img}")
                xTs_v = xTs[:].rearrange("p (t r) -> p t r", r=TS)
                xTs_l.append(xTs)
                xinI = xpool.tile([C, P], bf16, tag="x", name=f"xin{img}")
                nc.sync.dma_start(xinI[:], x_in[img])
                xin_l.append(xinI)
                psV = pV.tile([C, 512], f32, tag="psV", name=f"psV{img}")
                psV_l.append(psV)

                # ssq via PE: xsq = xin^2 (ACT), ones-col matmuls into the
                # image pair's shared psum bank (img%2 picks the 32-col half)
                if img % 2 == 0:
                    psS_l.append(pS.tile([C, 512], f32, tag="S",
                                         name=f"psumS{img // 2}"))
                psumS = psS_l[img // 2]
                base = (img % 2) * PT
                nc.sync.dma_start_transpose(xTs_v[:, :, 0:128], x_in[img])
                for ch in range(NCH):
                    xsq = wpool.tile([C, CH], bf16, tag="xsq",
                                     name=f"xsq{img}_{ch}")
                    nc.scalar.activation(xsq[:],
                                         xinI[:, ch * CH:(ch + 1) * CH],
                                         Act.Square)
                    for j in range(TPC):
                        t = ch * TPC + j
                        nc.tensor.matmul(psumS[:, base + t:base + t + 1],
                                         xsq[:, j * 128:(j + 1) * 128], onesb,
                                         start=True, stop=True)

                # stats: invn = quake-rsqrt(ssq) on DVE (no ACT tables);
                # n = ssq*invn written straight into the xTs 129th column.
                # image 0 runs chunk-granular so its first softmax does not
                # wait for the whole-image preamble at kernel start.
                invcI = spool.tile([C, PT], f32, tag="invcI",
                                   name=f"invcI{img}")
                sh = spool.tile([C, PT], i32, tag="sh", name=f"sh{img}")
                tq = spool.tile([C, PT], f32, tag="tq", name=f"tq{img}")

                def quake(lo, hi):
                    ssqp = psumS[:, base + lo:base + hi]
                    shs = sh[:, lo:hi]
                    ys = invcI[:, lo:hi]
                    ts = tq[:, lo:hi]
                    nc.vector.tensor_scalar(shs, ssqp.bitcast(i32), 1, None,
                                            Alu.logical_shift_right)
                    nc.vector.tensor_tensor(ys.bitcast(i32),
                                            magic[:, lo:hi], shs,
                                            Alu.subtract)
                    for _ in range(2):
                        nc.vector.tensor_tensor(ts, ssqp, ys, Alu.mult)
                        nc.vector.tensor_tensor(ts, ts, ys, Alu.mult)
                        nc.vector.tensor_scalar(ts, ts, -0.5, 1.5,
                                                Alu.mult, Alu.add)
                        nc.vector.tensor_tensor(ys, ys, ts, Alu.mult)
                    nc.vector.tensor_tensor(
                        xTs_v[:, lo:hi, 128:129].rearrange("p t r -> p (t r)"),
                        ssqp, ys, Alu.mult)

                if img == 0:
                    quake(0, TPC)
                    quake(TPC, PT)
                else:
                    quake(0, PT)
                invc_l.append(invcI)

            def chunks(img):
                xTs = xTs_l[img]
                psV = psV_l[img]
                for ch in range(NCH):
                    xin = xin_l[img][:, ch * CH:(ch + 1) * CH]
                    invc = invc_l[img][:, ch * TPC:(ch + 1) * TPC]

                    psumL = pL.tile([C, TPC * K], f32, tag="L")
                    for j in range(TPC):
                        nc.tensor.matmul(psumL[:, j * K:(j + 1) * K],
                                         xin[:, j * 128:(j + 1) * 128], wT,
                                         start=True, stop=True)

                    # lu = raw*invn (DVE big op from psum)
                    lu = wpool.tile([C, TPC * K], f32, tag="lu")
                    nc.vector.tensor_tensor(
                        lu[:].rearrange("p (t k) -> p t k", k=K),
                        psumL[:].rearrange("p (t k) -> p t k", k=K),
                        invc.broadcast_to([C, TPC, K]), Alu.mult)
                    # ll = lu + b (Pool)
                    ll = wpool.tile([C, TPC * K], f32, tag="ll")
                    nc.gpsimd.tensor_tensor(ll[:], lu[:], b512, Alu.add)
                    # -max over k per (pixel, tile)
                    nmcol = spool.tile([C, TPC], f32, tag="nmcol")
                    nc.vector.tensor_reduce(
                        nmcol[:], ll[:].rearrange("p (t k) -> p t k", k=K),
                        axis=mybir.AxisListType.X, op=Alu.max, negate=True)
                    # dd = ll - m (Pool)
                    dd = wpool.tile([C, TPC * K], f32, tag="dd")
                    nc.gpsimd.tensor_tensor(
                        dd[:].rearrange("p (t k) -> p t k", k=K),
                        ll[:].rearrange("p (t k) -> p t k", k=K),
                        nmcol[:].broadcast_to([C, TPC, K]), Alu.add)
                    # ee = exp(dd) one big ACT op
                    ee = wpool.tile([C, TPC * K], bf16, tag="ee")
                    nc.scalar.activation(ee[:], dd[:], Act.Exp)
                    # scol = sumexp (DVE)
                    scol = spool.tile([C, TPC], f32, tag="scol")
                    nc.vector.tensor_reduce(
                        scol[:], ee[:].rearrange("p (t k) -> p t k", k=K),
                        axis=mybir.AxisListType.X, op=Alu.add)
                    gcol = spool.tile([C, TPC], f32, tag="gcol")
                    nc.vector.reciprocal(gcol[:], scol[:])
                    rcol = spool.tile([C, TPC], f32, tag="rcol")
                    nc.vector.tensor_tensor(rcol[:], invc, gcol[:], Alu.mult)
                    # a_r = ee * rcol (Pool, broadcast over k)
                    aa = wpool.tile([C, TPC * K], bf16, tag="aa")
                    nc.gpsimd.tensor_tensor(
                        aa[:].rearrange("p (t k) -> p t k", k=K),
                        ee[:].rearrange("p (t k) -> p t k", k=K),
                        rcol[:].broadcast_to([C, TPC, K]), Alu.mult)

                    # defer vlads one chunk so PE finds independent
                    # logits work ahead of them in its in-order queue
                    if len(pend_vlad) >= 1:
                        pend_vlad.pop(0)()

                    def mk(aa=aa, ch=ch, psV=psV, xTs=xTs):
                        first = ch == 0
                        last = ch == NCH - 1
                        for j in range(TPC):
                            t = ch * TPC + j
                            nc.tensor.matmul(psV[0:KE, 0:129],
                                             aa[:, j * K:j * K + KE],
                                             xTs[:, t * TS:t * TS + 129],
                                             start=(first and j == 0),
                                             stop=(last and j == TPC - 1))
                    pend_vlad.append(mk)

            nc.sync.dma_start(dbg_ext[0:1, 0:1], cstf[0:1, 0:1])
            preamble(0)
            for img in range(NIMG):
                if img + 1 < NIMG:
                    preamble(img + 1)
                chunks(img)

            for fn in pend_vlad:
                fn()

            # ---- batched per-image tails ----
            ssqk4 = spool.tile([C, NIMG], f32, tag="ssqk4")
            vks = []
            for img in range(NIMG):
                psV = psV_l[img]
                negs = spool.tile([KE, 1], f32, tag="negs", name=f"negs{img}")
                nc.vector.tensor_scalar_mul(negs[:], psV[0:KE, 128:129], -1.0)
                vk = fpool.tile([KE, C], f32, tag="vk", name=f"vk{img}")
                nc.vector.scalar_tensor_tensor(vk[:], cen, negs[:],
                                               psV[0:KE, 0:C],
                                               Alu.mult, Alu.add)
                vks.append(vk)
            psA = pL.tile([C, TPC * K], f32, tag="L", name="psA")
            for img in range(NIMG):
                nc.tensor.matmul(psA[:, img * 128:img * 128 + KE], vks[img][:],
                                 identF[0:KE, 0:KE],
                                 is_transpose=True, start=True, stop=True)
                trash_a = fpool.tile([C, KE], f32, tag="tra", name=f"tra{img}")
                nc.scalar.activation(trash_a[:],
                                     psA[:, img * 128:img * 128 + KE],
                                     Act.Square,
                                     accum_out=ssqk4[:, img:img + 1])
            shk = spool.tile([C, NIMG], i32, tag="shk")
            nc.vector.tensor_scalar(shk[:], ssqk4[:].bitcast(i32), 1, None,
                                    Alu.logical_shift_right)
            yk = spool.tile([C, NIMG], f32, tag="yk")
            nc.vector.tensor_tensor(yk[:].bitcast(i32), magic[:, 0:NIMG],
                                    shk[:], Alu.subtract)
            tk = spool.tile([C, NIMG], f32, tag="tk")
            for _ in range(2):
                nc.vector.tensor_tensor(tk[:], ssqk4[:], yk[:], Alu.mult)
                nc.vector.tensor_tensor(tk[:], tk[:], yk[:], Alu.mult)
                nc.vector.tensor_scalar(tk[:], tk[:], -0.5, 1.5,
                                        Alu.mult, Alu.add)
                nc.vector.tensor_tensor(yk[:], yk[:], tk[:], Alu.mult)
            comb = spool.tile([C, NIMG], f32, tag="comb")
            nc.vector.tensor_scalar(comb[:], yk[:], 0.08838834764831845, None,
                                    Alu.mult)
            psB = pL.tile([C, TPC * K], f32, tag="L", name="psB")
            for img in range(NIMG):
                vnT = fpool.tile([C, KE], f32, tag="vnT", name=f"vnT{img}")
                nc.vector.tensor_scalar(vnT[:],
                                        psA[:, img * 128:img * 128 + KE],
                                        comb[:, img:img + 1], None, Alu.mult)
                nc.tensor.matmul(psB[0:KE, img * 128:img * 128 + C], vnT[:],
                                 identF,
                                 is_transpose=True, start=True, stop=True)
                ob = fpool.tile([KE, C], f32, tag="ob", name=f"ob{img}")
                nc.scalar.activation(ob[:],
                                     psB[0:KE, img * 128:img * 128 + C],
                                     Act.Copy)
                nc.sync.dma_start(out_ext[img], ob[:])

    nc.compile()
    return nc


def _get_nc():
    if "nc" not in _cache:
        _cache["nc"] = _build()
    return _cache["nc"]


def _make_in_maps(x, conv_w, conv_b, centroids, n_cores=8):
    x = np.asarray(x, dtype=np.float32)
    conv_w = np.asarray(conv_w, dtype=np.float32)
    conv_b = np.asarray(conv_b, dtype=np.float32)
    centroids = np.asarray(centroids, dtype=np.float32)

    N = x.shape[0]
    per = N // n_cores
    assert per == NIMG

    xr = np.ascontiguousarray(
        x.reshape(N, C, P).astype(ml_dtypes.bfloat16))

    cstb = np.ascontiguousarray(conv_w.T.astype(ml_dtypes.bfloat16))

    cstf = np.zeros((C, 840), dtype=np.float32)
    cstf[:, 0:K] = conv_b[None, :]
    cstf[:, 64:192] = np.eye(C, dtype=np.float32)
    cstf[0:KE, 192:320] = centroids[:KE]
    cstf[:, 320] = -0.5 * np.log(128.0)
    cstf[:, 328:328 + 512] = np.tile(conv_b, 8)[None, :]

    in_maps = []
    for i in range(n_cores):
        in_maps.append({
            "x": np.ascontiguousarray(xr[i * per:(i + 1) * per]),
            "cstb": cstb,
            "cstf": cstf,
        })
    return in_maps


def kernel(x, conv_w, conv_b, centroids):
    from concourse.bass_utils import run_bass_kernel_spmd

    n_cores = 8
    per = np.asarray(x).shape[0] // n_cores
    in_maps = _make_in_maps(x, conv_w, conv_b, centroids, n_cores)

    nc = _get_nc()
    res = run_bass_kernel_spmd(nc, in_maps, list(range(n_cores)))
    outs = [np.asarray(r["out"]).reshape(per, KE * C) for r in res.results]
    return np.concatenate(outs, axis=0)


if __name__ == "__main__":
    rng = np.random.default_rng(0)
    x = rng.standard_normal((32, C, 64, 64), dtype=np.float32)
    w = rng.standard_normal((K, C), dtype=np.float32)
    b = rng.standard_normal((K,), dtype=np.float32)
    c = rng.random((K, C), dtype=np.float32)
    out = kernel(x=x, conv_w=w, conv_b=b, centroids=c)
    print(out.shape, out.dtype)


# revision 56
# speedup vs baseline: 1.0048x; 1.0048x over previous
"""NetVLAD Trainium2 kernel — data-parallel over N across 8 cores, bf16.

Per core: 4 images [C=128, P=4096], host-cast to bf16.  x is loaded twice
per image: plain [c, p] (logits stationaries) and via one HWDGE
DMA-transpose [p, c] straight into SBUF (no PE transpose, no psum evict).

Per-image preamble (pipelined one image ahead):
  PE:   ssq via 32 ones-column matmuls (stationary = xsq tile from ACT
        Square of xin) -> psum [p, 32 cols]; decoupled from the transpose.
  DVE:  invn via Quake rsqrt (int bitcast + shift + 2 Newton steps) so
        exp is the ONLY ACT table function -> a single ACT_TABLE_LOAD for
        the whole kernel; n = ssq*invn lands directly in the xTs 129th
        column (stride-144 tiles, 32B-aligned).
Per 1024-px chunk (8 tiles):
  PE:   8 logits matmuls (stationary x_tile bf16, moving wT 64 cols).
  DVE:  lu = raw*invn (from psum); -max reduce (negate=True); sumexp
        reduce; reciprocal; rcol.
  Pool: ll = lu + b512; dd = ll - m (broadcast); a_r = ee * rcol.
  ACT:  ee = exp(dd) as one big [p,512] op.
  PE:   8 vlad matmuls (stationary a_r[:, :56], moving [xT | n] 129 cols),
        accumulated into psum [56, 129]; emission deferred one chunk so
        the in-order PE queue always has independent logits work first.
Batched tails: vk = term1 - s*cen, PE transpose, intra-norm via Quake
rsqrt * 1/sqrt(128) (the global norm is exactly 1/sqrt(128) because every
intra-normalized column has unit norm), transpose back, DMA.
"""

import sys

for _p in ("/opt/trn_rl_repo",):
    if _p not in sys.path:
        sys.path.insert(0, _p)

import numpy as np
import ml_dtypes

NIMG = 4      # images per core
C = 128
K = 64
KE = 56
P = 4096
TPC = 8       # 128-px tiles per chunk
CH = TPC * 128
NCH = P // CH  # 4 chunks per image
TS = 144      # xTs per-tile stride (bf16 elems); 288B = 32B-aligned

_cache = {}


def _build():
    import concourse.mybir as mybir
    from concourse import bacc, tile

    f32 = mybir.dt.float32
    i32 = mybir.dt.int32
    bf16 = mybir.dt.bfloat16
    Alu = mybir.AluOpType
    Act = mybir.ActivationFunctionType

    nc = bacc.Bacc()
    x_in = nc.declare_dram_parameter("x", [NIMG, C, P], bf16, isOutput=False)
    cb_in = nc.declare_dram_parameter("cstb", [C, K], bf16, isOutput=False)
    # f32 consts: b64 [0:64] | identF [64:192] | cen rows0:56 [192:320]
    # | ln(1/sqrt(128)) [320]
    cf_in = nc.declare_dram_parameter("cstf", [C, 840], f32, isOutput=False)
    out_ext = nc.declare_dram_parameter("out", [NIMG, KE, C], f32, isOutput=True)
    dbg_ext = nc.declare_dram_parameter("dbg", [C, 704], f32, isOutput=True)

    with tile.TileContext(nc) as tc:
        with (
            tc.tile_pool(name="const", bufs=1) as cpool,
            tc.tile_pool(name="xin", bufs=2) as xpool,
            tc.tile_pool(name="xts", bufs=2) as tpool,
            tc.tile_pool(name="work", bufs=4) as wpool,
            tc.tile_pool(name="stats", bufs=6) as spool,
            tc.tile_pool(name="fin", bufs=2) as fpool,
            tc.tile_pool(name="psL", bufs=2, space="PSUM") as pL,
            tc.tile_pool(name="psV", bufs=4, space="PSUM") as pV,
            tc.tile_pool(name="psS", bufs=2, space="PSUM") as pS,
        ):
            wT = cpool.tile([C, K], bf16, tag="wT")
            onescb = cpool.tile([C, 1], bf16, tag="onescb")
            cstf = cpool.tile([C, 840], f32, tag="cstf")
            nc.sync.dma_start(wT[:], cb_in[:])
            nc.vector.memset(onescb[:], 1.0)
            magic = cpool.tile([C, NCH * TPC], i32, tag="magic")
            nc.vector.memset(magic[:], 0x5f3759df)
            onesb = onescb[:]
            nc.sync.dma_start(cstf[:], cf_in[:])
            b64 = cstf[:, 0:K]
            identF = cstf[:, 64:192]
            cen = cstf[0:KE, 192:320]
            gnl = cstf[:, 320:321]
            b512 = cstf[:, 328:328 + TPC * K]

            PT = NCH * TPC  # 32 pixel tiles per image

            xTs_l, invc_l, xin_l, psV_l = [], [], [], []
            pend_vlad = []

            psS_l = []

            def preamble(img):
                xTs = tpool.tile([C, PT * TS], bf16, tag="xTs",
                                 name=f"xTs{img}")
                xTs_v = xTs[:].rearrange("p (t r) -> p t r", r=TS)
                xTs_l.append(xTs)
                xinI = xpool.tile([C, P], bf16, tag="x", name=f"xin{img}")
                nc.sync.dma_start(xinI[:], x_in[img])
                xin_l.append(xinI)
                psV = pV.tile([C, 512], f32, tag="psV", name=f"psV{img}")
                psV_l.append(psV)

                # ssq via PE: xsq = xin^2 (ACT), ones-col matmuls into the
                # image pair's shared psum bank (img%2 picks the 32-col half)
                if img % 2 == 0:
                    psS_l.append(pS.tile([C, 512], f32, tag="S",
                                         name=f"psumS{img // 2}"))
                psumS = psS_l[img // 2]
                base = (img % 2) * PT
                nc.sync.dma_start_transpose(xTs_v[:, :, 0:128], x_in[img])
                for ch in range(NCH):
                    xsq = wpool.tile([C, CH], bf16, tag="xsq",
                                     name=f"xsq{img}_{ch}")
                    nc.scalar.activation(xsq[:],
                                         xinI[:, ch * CH:(ch + 1) * CH],
                                         Act.Square)
                    for j in range(TPC):
                        t = ch * TPC + j
                        nc.tensor.matmul(psumS[:, base + t:base + t + 1],
                                         xsq[:, j * 128:(j + 1) * 128], onesb,
                                         start=True, stop=True)

                # stats: invn = quake-rsqrt(ssq) on DVE (no ACT tables);
                # n = ssq*invn written straight into the xTs 129th column.
                # image 0 runs chunk-granular so its first softmax does not
                # wait for the whole-image preamble at kernel start.
                invcI = spool.tile([C, PT], f32, tag="invcI",
                                   name=f"invcI{img}")
                sh = spool.tile([C, PT], i32, tag="sh", name=f"sh{img}")
                tq = spool.tile([C, PT], f32, tag="tq", name=f"tq{img}")

                def quake(lo, hi):
                    ssqp = psumS[:, base + lo:base + hi]
                    shs = sh[:, lo:hi]
                    ys = invcI[:, lo:hi]
                    ts = tq[:, lo:hi]
                    nc.vector.tensor_scalar(shs, ssqp.bitcast(i32), 1, None,
                                            Alu.logical_shift_right)
                    nc.vector.tensor_tensor(ys.bitcast(i32),
                                            magic[:, lo:hi], shs,
                                            Alu.subtract)
                    for _ in range(2):
                        nc.vector.tensor_tensor(ts, ssqp, ys, Alu.mult)
                        nc.vector.tensor_tensor(ts, ts, ys, Alu.mult)
                        nc.vector.tensor_scalar(ts, ts, -0.5, 1.5,
                                                Alu.mult, Alu.add)
                        nc.vector.tensor_tensor(ys, ys, ts, Alu.mult)
                    nc.vector.tensor_tensor(
                        xTs_v[:, lo:hi, 128:129].rearrange("p t r -> p (t r)"),
                        ssqp, ys, Alu.mult)

                if img == 0:
                    quake(0, TPC)
                    quake(TPC, PT)
                else:
                    quake(0, PT)
                invc_l.append(invcI)

            def chunks(img):
                xTs = xTs_l[img]
                psV = psV_l[img]
                for ch in range(NCH):
                    xin = xin_l[img][:, ch * CH:(ch + 1) * CH]
                    invc = invc_l[img][:, ch * TPC:(ch + 1) * TPC]

                    psumL = pL.tile([C, TPC * K], f32, tag="L")
                    for j in range(TPC):
                        nc.tensor.matmul(psumL[:, j * K:(j + 1) * K],
                                         xin[:, j * 128:(j + 1) * 128], wT,
                                         start=True, stop=True)

                    # lu = raw*invn (DVE big op from psum)
                    lu = wpool.tile([C, TPC * K], f32, tag="lu")
                    nc.vector.tensor_tensor(
                        lu[:].rearrange("p (t k) -> p t k", k=K),
                        psumL[:].rearrange("p (t k) -> p t k", k=K),
                        invc.broadcast_to([C, TPC, K]), Alu.mult)
                    # ll = lu + b (Pool)
                    ll = wpool.tile([C, TPC * K], f32, tag="ll")
                    nc.gpsimd.tensor_tensor(ll[:], lu[:], b512, Alu.add)
                    # -max over k per (pixel, tile)
                    nmcol = spool.tile([C, TPC], f32, tag="nmcol")
                    nc.vector.tensor_reduce(
                        nmcol[:], ll[:].rearrange("p (t k) -> p t k", k=K),
                        axis=mybir.AxisListType.X, op=Alu.max, negate=True)
                    # dd = ll - m (Pool)
                    dd = wpool.tile([C, TPC * K], f32, tag="dd")
                    nc.gpsimd.tensor_tensor(
                        dd[:].rearrange("p (t k) -> p t k", k=K),
                        ll[:].rearrange("p (t k) -> p t k", k=K),
                        nmcol[:].broadcast_to([C, TPC, K]), Alu.add)
                    # ee = exp(dd) one big ACT op
                    ee = wpool.tile([C, TPC * K], bf16, tag="ee")
                    nc.scalar.activation(ee[:], dd[:], Act.Exp)
                    # scol = sumexp (DVE)
                    scol = spool.tile([C, TPC], f32, tag="scol")
                    nc.vector.tensor_reduce(
                        scol[:], ee[:].rearrange("p (t k) -> p t k", k=K),
                        axis=mybir.AxisListType.X, op=Alu.add)
                    gcol = spool.tile([C, TPC], f32, tag="gcol")
                    nc.vector.reciprocal(gcol[:], scol[:])
                    rcol = spool.tile([C, TPC], f32, tag="rcol")
                    nc.vector.tensor_tensor(rcol[:], invc, gcol[:], Alu.mult)
                    # a_r = ee * rcol (Pool, broadcast over k)
                    aa = wpool.tile([C, TPC * K], bf16, tag="aa")
                    nc.gpsimd.tensor_tensor(
                        aa[:].rearrange("p (t k) -> p t k", k=K),
                        ee[:].rearrange("p (t k) -> p t k", k=K),
                        rcol[:].broadcast_to([C, TPC, K]), Alu.mult)

                    # defer vlads one chunk so PE finds independent
                    # logits work ahead of them in its in-order queue
                    if len(pend_vlad) >= 1:
                        pend_vlad.pop(0)()

                    def mk(aa=aa, ch=ch, psV=psV, xTs=xTs):
                        first = ch == 0
                        last = ch == NCH - 1
                        for j in range(TPC):
                            t = ch * TPC + j
                            nc.tensor.matmul(psV[0:KE, 0:129],
                                             aa[:, j * K:j * K + KE],
                                             xTs[:, t * TS:t * TS + 129],
                                             start=(first and j == 0),
                                             stop=(last and j == TPC - 1))
                    pend_vlad.append(mk)

            nc.sync.dma_start(dbg_ext[0:1, 0:1], cstf[0:1, 0:1])
            preamble(0)
            for img in range(NIMG):
                if img + 1 < NIMG:
                    preamble(img + 1)
                chunks(img)

            for fn in pend_vlad:
                fn()

            # ---- batched per-image tails ----
            ssqk4 = spool.tile([C, NIMG], f32, tag="ssqk4")
            vks = []
            for img in range(NIMG):
                psV = psV_l[img]
                negs = spool.tile([KE, 1], f32, tag="negs", name=f"negs{img}")
                nc.vector.tensor_scalar_mul(negs[:], psV[0:KE, 128:129], -1.0)
                vk = fpool.tile([KE, C], f32, tag="vk", name=f"vk{img}")
                nc.vector.scalar_tensor_tensor(vk[:], cen, negs[:],
                                               psV[0:KE, 0:C],
                                               Alu.mult, Alu.add)
                vks.append(vk)
            psA = pL.tile([C, TPC * K], f32, tag="L", name="psA")
            for img in range(NIMG):
                nc.tensor.matmul(psA[:, img * 128:img * 128 + KE], vks[img][:],
                                 identF[0:KE, 0:KE],
                                 is_transpose=True, start=True, stop=True)
                trash_a = fpool.tile([C, KE], f32, tag="tra", name=f"tra{img}")
                nc.scalar.activation(trash_a[:],
                                     psA[:, img * 128:img * 128 + KE],
                                     Act.Square,
                                     accum_out=ssqk4[:, img:img + 1])
            shk = spool.tile([C, NIMG], i32, tag="shk")
            nc.vector.tensor_scalar(shk[:], ssqk4[:].bitcast(i32), 1, None,
                                    Alu.logical_shift_right)
            yk = spool.tile([C, NIMG], f32, tag="yk")
            nc.vector.tensor_tensor(yk[:].bitcast(i32), magic[:, 0:NIMG],
                                    shk[:], Alu.subtract)
            tk = spool.tile([C, NIMG], f32, tag="tk")
            for _ in range(2):
                nc.vector.tensor_tensor(tk[:], ssqk4[:], yk[:], Alu.mult)
                nc.vector.tensor_tensor(tk[:], tk[:], yk[:], Alu.mult)
                nc.vector.tensor_scalar(tk[:], tk[:], -0.5, 1.5,
                                        Alu.mult, Alu.add)
                nc.vector.tensor_tensor(yk[:], yk[:], tk[:], Alu.mult)
            comb = spool.tile([C, NIMG], f32, tag="comb")
            nc.vector.tensor_scalar(comb[:], yk[:], 0.08838834764831845, None,
                                    Alu.mult)
            psB = pL.tile([C, TPC * K], f32, tag="L", name="psB")
            for img in range(NIMG):
                vnT = fpool.tile([C, KE], f32, tag="vnT", name=f"vnT{img}")
                nc.vector.tensor_scalar(vnT[:],
                                        psA[:, img * 128:img * 128 + KE],
                                        comb[:, img:img + 1], None, Alu.mult)
                nc.tensor.matmul(psB[0:KE, img * 128:img * 128 + C], vnT[:],
                                 identF,
                                 is_transpose=True, start=True, stop=True)
                ob = fpool.tile([KE, C], f32, tag="ob", name=f"ob{img}")
                nc.scalar.activation(ob[:],
                                     psB[0:KE, img * 128:img * 128 + C],
                                     Act.Copy)
                nc.sync.dma_start(out_ext[img], ob[:])

    nc.compile()
    return nc


def _get_nc():
    if "nc" not in _cache:
        _cache["nc"] = _build()
    return _cache["nc"]


def _make_in_maps(x, conv_w, conv_b, centroids, n_cores=8):
    x = np.asarray(x, dtype=np.float32)
    conv_w = np.asarray(conv_w, dtype=np.float32)
    conv_b = np.asarray(conv_b, dtype=np.float32)
    centroids = np.asarray(centroids, dtype=np.float32)

    N = x.shape[0]
    per = N // n_cores
    assert per == NIMG

    xr = np.ascontiguousarray(
        x.reshape(N, C, P).astype(ml_dtypes.bfloat16))

    cstb = np.ascontiguousarray(conv_w.T.astype(ml_dtypes.bfloat16))

    cstf = np.zeros((C, 840), dtype=np.float32)
    cstf[:, 0:K] = conv_b[None, :]
    cstf[:, 64:192] = np.eye(C, dtype=np.float32)
    cstf[0:KE, 192:320] = centroids[:KE]
    cstf[:, 320] = -0.5 * np.log(128.0)
    cstf[:, 328:328 + 512] = np.tile(conv_b, 8)[None, :]

    in_maps = []
    for i in range(n_cores):
        in_maps.append({
            "x": np.ascontiguousarray(xr[i * per:(i + 1) * per]),
            "cstb": cstb,
            "cstf": cstf,
        })
    return in_maps


def kernel(x, conv_w, conv_b, centroids):
    from concourse.bass_utils import run_bass_kernel_spmd

    n_cores = 8
    per = np.asarray(x).shape[0] // n_cores
    in_maps = _make_in_maps(x, conv_w, conv_b, centroids, n_cores)

    nc = _get_nc()
    res = run_bass_kernel_spmd(nc, in_maps, list(range(n_cores)))
    outs = [np.asarray(r["out"]).reshape(per, KE * C) for r in res.results]
    return np.concatenate(outs, axis=0)


if __name__ == "__main__":
    rng = np.random.default_rng(0)
    x = rng.standard_normal((32, C, 64, 64), dtype=np.float32)
    w = rng.standard_normal((K, C), dtype=np.float32)
    b = rng.standard_normal((K,), dtype=np.float32)
    c = rng.random((K, C), dtype=np.float32)
    out = kernel(x=x, conv_w=w, conv_b=b, centroids=c)
    print(out.shape, out.dtype)
